# revision 16
# baseline (speedup 1.0000x reference)
"""Trainium2 Bass kernel for nn_Decoder_1692217114985 (continuous transpose-conv decoder).

Math (see the reference):
  integ = FF(weights)                         # [B=64, K=400] per-stride integrals
  kval[f,n,k] = MLP_f(grid[n] - center[k])    # masked to the 0.15-window
  out = sigmoid(einsum('fnk,bk->bnf', kval, integ))

Key optimization over the dense baseline: the 0.15 filter window only spans
3-4 center strides per axis, so each grid point's active centers live in a
64-wide contiguous band of k = 20*ix+iy indices starting at k0 = 20*bx+by.
The host sorts points by k0 and gives each core 256 points whose bands all
fit in ONE 120-row k-window [c0, c0+120).  Each core then runs the dense
pair-MLP against only its 120 window rows (30k pairs) instead of all 400
centers (102k pairs), with per-core-sliced ffw3/ffb3 and host-precomputed
MLP inputs making the program pure SPMD.  Outputs are un-permuted on the
host.

Per-core layout:
  - The pair-MLP rhs (window-local coords, fp16) and the exact fp32 window
    mask are pure functions of `grid`, so the host precomputes both; the
    device spends no time on coordinate broadcasts or mask compares.
  - All matmul datapaths run in float16; PSUM accumulation is fp32.
  - FF MLP computed transposed (features on partitions, batch on free dim)
    producing integT [120(k-window), 64] directly from the sliced ffw3, and
    is interleaved with the early pair-MLP chunks on the PE queue.
  - The pair MLP (2->20->20->1, x2 fields) is evaluated over the window with
    a 3-way block-diagonal packing: 3 k-slabs of 40 rows stacked on the
    contraction dim, so each PE column evaluates 3 (point, center) pairs.
    Columns are (j, n) with j in [0,40), n in [0,256): slab s row = 40s+j.
  - Layer-2 weights carry an extra constant-1 unit per slice so the layer-3
    bias rides through the matmul; layer-1/2 biases enter via the per-partition
    bias ports (relu work alternates ScalarE/VectorE).
  - Layer-3 outputs are stacked 4 chunks per PSUM tile via tile_position=
    (0,32q), copied once per tile to SBUF, bounced through a DRAM staging
    buffer, and gathered back into the [120, 2*256] kval tile with mixed-radix
    strided DMAs (DRAM-side access patterns are unconstrained), one DMA per
    (stag-tile, slab) covering both fields, pipelined against later chunks.
  - kval is masked with the host-computed window indicator and contracted
    against integT on the PE, then pushed through sigmoid.
"""

import numpy as np
from contextlib import ExitStack

import concourse.bacc as bacc
import concourse.bass as bass
import concourse.tile as tile
from concourse import mybir
from concourse.bass_utils import run_bass_kernel_spmd

F32 = mybir.dt.float32
F16 = mybir.dt.float16
AF = mybir.ActivationFunctionType
OP = mybir.AluOpType

B, H, N, F, KH = 64, 256, 2048, 2, 20
K = 400
NCORES = 8
NLOC = N // NCORES          # 256 grid points per core
W = 120                     # k-window rows per core (3 slabs x 40)
S = 3                       # packed k-slabs
JW = 40                     # j (within-slab row) count; slab s row = 40s+j
NCOLS = JW * NLOC           # 10240 pair columns = 20 chunks exactly
NCHUNK = 20
FILT = 0.15

# f16 constant blob column offsets
C_W1P, C_W2P, C_W3P = 0, 120, 243
C_FFW1A, C_FFW1B, C_FFW2 = 275, 395, 515
C_FFW3A, C_FFW3B = 755, 875
C_WT0, C_WT1 = 995, 1059
C16 = 1123

LAST_RESULTS = None          # BassKernelResults of the most recent run


def _build_nc():
    nc = bacc.Bacc("TRN2", name="decoder")

    # ---- IO ----
    d_rhs = nc.dram_tensor("rhs", [38, NCOLS], F16, kind="ExternalInput")
    d_msk = nc.dram_tensor("msk", [W, F * NLOC], F16, kind="ExternalInput")
    d_c16 = nc.dram_tensor("c16", [128, C16], F16, kind="ExternalInput")
    d_c32 = nc.dram_tensor("c32", [128, 8], F32, kind="ExternalInput")
    d_out = nc.dram_tensor("out", [B, NLOC, F], F32, kind="ExternalOutput")

    with tile.TileContext(nc) as tc, ExitStack() as ctx:
        consts = ctx.enter_context(tc.tile_pool(name="consts", bufs=1))
        persist = ctx.enter_context(tc.tile_pool(name="persist", bufs=1))
        big = ctx.enter_context(tc.tile_pool(name="big", bufs=1))
        work = ctx.enter_context(tc.tile_pool(name="work", bufs=4))
        kvpool = ctx.enter_context(tc.tile_pool(name="kv", bufs=4))
        dramp = ctx.enter_context(tc.tile_pool(name="dramp", bufs=2, space="DRAM"))
        psum = ctx.enter_context(tc.tile_pool(name="psum", bufs=1, space="PSUM"))

        # preload the activation table (Tanh set also carries Relu/Sigmoid/
        # Identity) so no mid-kernel ~1.3us ACT_TABLE_LOAD fires
        actdum = consts.tile([1, 2], F32, tag="actdum")
        nc.vector.memset(actdum[0:1, 0:1], 0.0)
        nc.scalar.activation(actdum[0:1, 1:2], actdum[0:1, 0:1], AF.Tanh)

        # ---- load inputs; rhs split into 4 pieces so chunk 0 starts early ----
        rhs1 = big.tile([38, NCOLS], F16, tag="rhs1")
        rhs_e = (nc.sync, nc.gpsimd, nc.scalar, nc.sync)
        rsplit = [0, 2 * 512, 7 * 512, 13 * 512, NCOLS]
        for p in range(4):
            rhs_e[p].dma_start(out=rhs1[:, rsplit[p]:rsplit[p + 1]],
                               in_=d_rhs[:, rsplit[p]:rsplit[p + 1]])
        c16 = consts.tile([128, C16], F16, tag="c16")
        nc.scalar.dma_start(out=c16[:], in_=d_c16[:, :])
        c32 = consts.tile([128, 8], F32, tag="c32")
        nc.gpsimd.dma_start(out=c32[:], in_=d_c32[:, :])
        msk = consts.tile([W, F * NLOC], F16, tag="msk")
        nc.gpsimd.dma_start(out=msk[:], in_=d_msk[:, :])

        ffb3c = c32[:, 2:3]
        ffb1c = c32[0:120, 3:4]
        ffb2c = c32[0:120, 4:5]
        b1p = c32[0:120, 6:7]
        b2p = c32[0:123, 7:8]

        w2p = c16[0:120, C_W2P:C_W2P + 123]
        w3p = c16[0:123, C_W3P:C_W3P + 32]
        ffw1a = c16[0:128, C_FFW1A:C_FFW1A + 120]
        ffw1b = c16[0:128, C_FFW1B:C_FFW1B + 120]
        ffw2 = c16[0:120, C_FFW2:C_FFW2 + 240]
        ffw3a = c16[0:120, C_FFW3A:C_FFW3A + W]
        ffw3b = c16[0:120, C_FFW3B:C_FFW3B + W]
        wt0 = c16[0:128, C_WT0:C_WT0 + 64]
        wt1 = c16[0:128, C_WT1:C_WT1 + 64]

        # ---- FF MLP pieces (emitted interleaved with pair-MLP chunks) ----
        h1ff = work.tile([120, B], F16, tag="h1ff")
        h2ff = work.tile([120, 2 * B], F16, tag="h2ff")
        integT = persist.tile([128, B], F16, tag="integT")

        def ff_l1():
            ps = psum.tile([120, B], F32, tag="ps2", bufs=2, name="ffp")
            nc.tensor.matmul(ps[:], ffw1a, wt0, start=True, stop=False)
            nc.tensor.matmul(ps[:], ffw1b, wt1, start=False, stop=True)
            nc.scalar.activation(h1ff[:], ps[:], AF.Tanh, bias=ffb1c)
            return ps

        def ff_l2():
            ps = psum.tile([120, 2 * B], F32, tag="ps2", bufs=2, name="ffp")
            nc.tensor.matmul(ps[:, 0:B], ffw2[:, 0:120], h1ff[:],
                             start=True, stop=True)
            nc.tensor.matmul(ps[:, B:2 * B], ffw2[:, 120:240], h1ff[:],
                             start=True, stop=True)
            nc.scalar.activation(h2ff[:], ps[:], AF.Tanh, bias=ffb2c)
            return ps

        def ff_l3():
            ps = psum.tile([120, B], F32, tag="ps2", bufs=2, name="ffp")
            nc.tensor.matmul(ps[:], ffw3a, h2ff[:, 0:B], start=True, stop=False)
            nc.tensor.matmul(ps[:], ffw3b, h2ff[:, B:2 * B], start=False, stop=True)
            nc.scalar.activation(integT[0:120, :], ps[:], AF.Identity,
                                 bias=ffb3c[0:120, :])
            return ps

        ff_stages = [ff_l1, ff_l2, ff_l3]

        # ---- pipelined 3-layer pair MLP over 40 256-column units ----
        # 256-col granularity keeps the dependency chain (L1 -> relu -> L2 ->
        # relu -> L3) shorter than 4 units of PE work, so the PE queue never
        # stalls and the tensor engine ramps to its full clock.  Relu work is
        # split across ScalarE/VectorE with one op per engine per step.
        # Layer-3 outputs stack 8 units deep in a PSUM tile via tile_position,
        # are copied once per tile to SBUF (split halves on both engines),
        # bounced into DRAM staging, and gathered per-tile into kval with
        # mixed-radix strided DMAs while later units still run.
        stag = dramp.tile([5, 128, 512], F16, tag="stag")
        kval = persist.tile([W, F * NLOC], F16, tag="kval")
        NU = 2 * NCHUNK
        ps1s, ps2s, ps3s = {}, {}, {}
        g_engines = (nc.sync, nc.sync, nc.gpsimd, nc.gpsimd, nc.sync, nc.sync)

        ps1t, ps2t = {}, {}

        def emit_l1(u):
            if u % 8 == 0:
                ps3s[u // 8] = psum.tile([128, 512], F32, tag="ps3",
                                         bufs=2, name="ps3")
            csl = slice(u * 256, (u + 1) * 256)
            # PSUM banks are 2KB: allocate one [*, 512] bank per unit pair
            # and hand out 256-col halves
            if u % 2 == 0:
                ps1t[u // 2] = psum.tile([120, 512], F32, tag="ps1", bufs=4, name="ps1t")
            ps1 = ps1t[u // 2][:, 256 * (u % 2):256 * (u % 2) + 256]
            r = 32 * ((u // 2) % 2)   # row-strip: L1's K is tiny, so odd/even
            nc.tensor.matmul(ps1, c16[r:r + 6, C_W1P:C_W1P + 120],
                             rhs1[r:r + 6, csl],
                             start=True, stop=True, tile_position=(r, 0))
            ps1s[u] = ps1

        def emit_l2(u):
            ps1 = ps1s.pop(u)
            h1 = work.tile([120, 256], F16, tag="h1")
            if u % 2 == 0:
                nc.scalar.activation(h1[:], ps1, AF.Relu, bias=b1p)
            else:
                nc.vector.tensor_scalar(h1[:], ps1, b1p, 0.0, OP.add, OP.max)
            if u % 2 == 0:
                ps2t[u // 2] = psum.tile([123, 512], F32, tag="ps2", bufs=2, name="ps2t")
            ps2 = ps2t[u // 2][:, 256 * (u % 2):256 * (u % 2) + 256]
            nc.tensor.matmul(ps2, w2p, h1[:], start=True, stop=True)
            ps2s[u] = ps2

        def emit_relu2(u):
            ps2 = ps2s.pop(u)
            h2 = work.tile([123, 256], F16, tag="h2")
            if u % 2 == 0:
                nc.vector.tensor_scalar(h2[:], ps2, b2p, 0.0, OP.add, OP.max)
            else:
                nc.scalar.activation(h2[:], ps2, AF.Relu, bias=b2p)
            ps2s[u] = h2

        def emit_l3(u):
            h2 = ps2s.pop(u)
            ch, hf = divmod(u, 2)
            t, q = divmod(ch, 4)
            nc.tensor.matmul(ps3s[t][32 * q:32 * q + 32, 256 * hf:256 * hf + 256],
                             w3p, h2[:], start=True, stop=True,
                             tile_position=(0, 32 * q))
            if u % 8 == 7:
                kvp = kvpool.tile([128, 512], F16, tag="kvp")
                ps3 = ps3s.pop(t)
                nc.scalar.copy(kvp[0:64, :], ps3[0:64, :])
                nc.vector.tensor_copy(kvp[64:128, :], ps3[64:128, :])
                nc.gpsimd.dma_start(out=stag[t, :, :], in_=kvp[:])
                # gather this tile's rows for all (slab, field) while the
                # remaining units run: col c = j*256+n -> chunk j//2 at
                # strip 32*((j//2)%4), pos (j%2)*256+n; j = 8t+2b+e
                st = stag[:]
                for s in range(S):
                    for f in range(F):
                        src = bass.AP(
                            tensor=st.tensor,
                            offset=st.offset + t * 65536 + 512 * (2 * s + f),
                            ap=[[16384, 4], [256, 2], [1, NLOC]])
                        g_engines[2 * s + f].dma_start(
                            out=kval[JW * s + 8 * t:JW * s + 8 * t + 8,
                                     f * NLOC:(f + 1) * NLOC],
                            in_=src)

        # unit pipeline: the PE runs L1(u), L2(u-2), L3(u-4) each step while
        # the relus of the units in between complete on ScalarE/VectorE; FF
        # stages slot into the PE queue at wide spacings
        for step in range(NU + 4):
            if step < NU:
                emit_l1(step)
            if step in (2, 12, 22):
                ff_stages[(step - 2) // 10]()
            if 2 <= step and step - 2 < NU:
                emit_l2(step - 2)
            if 3 <= step and step - 3 < NU:
                emit_relu2(step - 3)
            if 4 <= step and step - 4 < NU:
                emit_l3(step - 4)

        # ---- mask, contract against integT, sigmoid, store ----
        outsb = persist.tile([B, NLOC, F], F32, tag="outsb")
        nc.vector.tensor_tensor(kval[:], kval[:], msk[:], OP.mult)
        for f in range(F):
            fsl = slice(f * NLOC, (f + 1) * NLOC)
            psF = psum.tile([B, NLOC], F32, tag="ps2", bufs=2)
            nc.tensor.matmul(psF[:], integT[0:W, :], kval[:, fsl],
                             start=True, stop=True)
            nc.scalar.activation(outsb[:, :, f], psF[:], AF.Sigmoid)
        nc.sync.dma_start(out=d_out[:, :, :], in_=outsb[:])

    nc.finalize()
    return nc


_NC_CACHE = None


def _get_nc():
    global _NC_CACHE
    if _NC_CACHE is None:
        _NC_CACHE = _build_nc()
    return _NC_CACHE


def _pack_shared(w):
    """Host-side packing of the grid-independent constants."""
    f32, f16 = np.float32, np.float16
    k_w1, k_b1 = w["k_w1"].astype(f32), w["k_b1"].astype(f32)
    k_w2, k_b2 = w["k_w2"].astype(f32), w["k_b2"].astype(f32)
    k_w3, k_b3 = w["k_w3"].astype(f32), w["k_b3"].astype(f32)
    w1p = np.zeros((38, 120), f32)
    b1p = np.zeros((120,), f32)
    w2p = np.zeros((120, 123), f32)
    b2p = np.zeros((123,), f32)
    w3p = np.zeros((123, 32), f32)
    for s in range(S):
        for f in range(F):
            o = s * 40 + f * 20
            for d in range(2):
                w1p[2 * s + d, o:o + 20] = k_w1[f, d]
                w1p[32 + 2 * s + d, o:o + 20] = k_w1[f, d]
            b1p[o:o + 20] = k_b1[f]
            w2p[o:o + 20, s * 41 + f * 20:s * 41 + f * 20 + 20] = k_w2[f]
            b2p[s * 41 + f * 20:s * 41 + f * 20 + 20] = k_b2[f]
            w3p[s * 41 + f * 20:s * 41 + f * 20 + 20, s * 2 + f] = k_w3[f, :, 0]
            w3p[s * 41 + 40, s * 2 + f] = k_b3[f, 0]
        b2p[s * 41 + 40] = 1.0

    c16 = np.zeros((128, C16), f16)
    c16[0:38, C_W1P:C_W1P + 120] = w1p.astype(f16)
    c16[0:120, C_W2P:C_W2P + 123] = w2p.astype(f16)
    c16[0:123, C_W3P:C_W3P + 32] = w3p.astype(f16)
    ffw1 = w["ff_w1"].astype(f16)
    c16[0:128, C_FFW1A:C_FFW1A + 120] = ffw1[0:128]
    c16[0:128, C_FFW1B:C_FFW1B + 120] = ffw1[128:256]
    c16[0:120, C_FFW2:C_FFW2 + 240] = w["ff_w2"].astype(f16)
    wT = np.ascontiguousarray(w["weights"].astype(f32).T).astype(f16)
    c16[0:128, C_WT0:C_WT0 + 64] = wT[0:128]
    c16[0:128, C_WT1:C_WT1 + 64] = wT[128:256]

    c32 = np.zeros((128, 8), f32)
    c32[0:120, 3] = w["ff_b1"].astype(f32)
    c32[0:120, 4] = w["ff_b2"].astype(f32)[0:120]
    c32[0:120, 5] = w["ff_b2"].astype(f32)[120:240]
    c32[0:120, 6] = b1p
    c32[0:123, 7] = b2p
    return c16, c32


def kernel(**inputs):
    global LAST_RESULTS
    f32, f16 = np.float32, np.float16
    nc = _get_nc()
    c16s, c32s = _pack_shared(inputs)
    grid = inputs["grid"].astype(f32)

    # exact fp32 active-window bases per point (replicates the reference's
    # fp32 center table and window comparisons bit-for-bit)
    g = (np.arange(20, dtype=f32) * f32(0.05)).astype(f32)
    lx = grid[:, 0:1] - g[None, :]   # [N, 20] exact fp32
    ly = grid[:, 1:2] - g[None, :]
    ax = (lx >= 0) & (lx <= f32(FILT))
    ay = (ly >= 0) & (ly <= f32(FILT))
    bx = np.minimum(ax.argmax(1), 16)
    by = np.minimum(ay.argmax(1), 16)
    ii = np.arange(20)[None, :]
    assert np.all(~ax | ((ii >= bx[:, None]) & (ii <= bx[:, None] + 3)))
    assert np.all(~ay | ((ii >= by[:, None]) & (ii <= by[:, None] + 3)))
    k0 = 20 * bx + by
    perm = np.argsort(k0, kind="stable")
    k0s = k0[perm]

    # padded ffw3/ffb3 for per-core window slicing
    ffw3p = np.zeros((240, 512), f32)
    ffw3p[:, :K] = inputs["ff_w3"].astype(f32)
    ffb3p = np.zeros((512,), f32)
    ffb3p[:K] = inputs["ff_b3"].astype(f32)

    in_maps = []
    for c in range(NCORES):
        sl = perm[c * NLOC:(c + 1) * NLOC]
        c0 = int(k0s[c * NLOC])
        assert int(k0s[(c + 1) * NLOC - 1]) - c0 <= W - 64, "window overflow"
        kk = c0 + np.arange(W)
        kix = np.minimum(kk // 20, 19)
        kiy = kk % 20
        # window-local fp32 coords [W, NLOC] (exact: same subtract as ref)
        wlx = grid[sl, 0][None, :] - g[kix][:, None]
        wly = grid[sl, 1][None, :] - g[kiy][:, None]
        inside = ((wlx >= 0) & (wlx <= f32(FILT)) &
                  (wly >= 0) & (wly <= f32(FILT)) &
                  (kk < K)[:, None])
        # pair-MLP rhs [38, 40*256] fp16: col (j, n), slab s rows 2s:2s+2,
        # replicated at 32+2s for the odd-chunk row strip
        rhs = np.zeros((38, NCOLS), f16)
        for s in range(S):
            rhs[2 * s] = wlx[JW * s:JW * s + JW].astype(f16).reshape(-1)
            rhs[2 * s + 1] = wly[JW * s:JW * s + JW].astype(f16).reshape(-1)
        rhs[32:38] = rhs[0:6]
        c16 = c16s.copy()
        c16[0:120, C_FFW3A:C_FFW3A + W] = ffw3p[0:120, c0:c0 + W].astype(f16)
        c16[0:120, C_FFW3B:C_FFW3B + W] = ffw3p[120:240, c0:c0 + W].astype(f16)
        c32 = c32s.copy()
        c32[0:W, 2] = ffb3p[c0:c0 + W]
        in_maps.append(dict(
            c16=c16, c32=c32, rhs=rhs,
            msk=np.concatenate([inside.astype(f16)] * F, axis=1),
        ))
    res = run_bass_kernel_spmd(nc, in_maps, core_ids=list(range(NCORES)))
    LAST_RESULTS = res
    out_sorted = np.concatenate([r["out"] for r in res.results], axis=1)
    out = np.empty_like(out_sorted)
    out[:, perm, :] = out_sorted
    return out


# revision 20
# speedup vs baseline: 1.1354x; 1.1354x over previous
"""Trainium2 Bass kernel for nn_Decoder_1692217114985 (continuous transpose-conv decoder).

Math (see the reference):
  integ = FF(weights)                         # [B=64, K=400] per-stride integrals
  kval[f,n,k] = MLP_f(grid[n] - center[k])    # masked to the 0.15-window
  out = sigmoid(einsum('fnk,bk->bnf', kval, integ))

Key optimization over the dense baseline: the 0.15 filter window only spans
3-4 center strides per axis, so each grid point's active centers live in a
64-wide contiguous band of k = 20*ix+iy indices starting at k0 = 20*bx+by.
The host sorts points by k0 and gives each core 256 points whose bands all
fit in ONE 120-row k-window [c0, c0+120).  Each core then runs the dense
pair-MLP against only its 120 window rows (30k pairs) instead of all 400
centers (102k pairs), with per-core-sliced ffw3/ffb3 and host-precomputed
MLP inputs making the program pure SPMD.  Outputs are un-permuted on the
host.

Per-core layout:
  - The pair-MLP rhs (window-local coords, fp16) and the exact fp32 window
    mask are pure functions of `grid`, so the host precomputes both; the
    device spends no time on coordinate broadcasts or mask compares.
  - All matmul datapaths run in float16; PSUM accumulation is fp32.
  - FF MLP computed transposed (features on partitions, batch on free dim)
    producing integT [120(k-window), 64] directly from the sliced ffw3, and
    is interleaved with the early pair-MLP chunks on the PE queue.
  - The pair MLP (2->20->20->1, x2 fields) is evaluated over the window with
    a 3-way block-diagonal packing: 3 k-slabs of 40 rows stacked on the
    contraction dim, so each PE column evaluates 3 (point, center) pairs.
    Columns are (j, n) with j in [0,40), n in [0,256): slab s row = 40s+j.
  - Layer-2 weights carry an extra constant-1 unit per slice so the layer-3
    bias rides through the matmul; layer-1/2 biases enter via the per-partition
    bias ports (relu work alternates ScalarE/VectorE).
  - Layer-3 outputs are stacked 4 chunks per PSUM tile via tile_position=
    (0,32q), copied once per tile to SBUF, bounced through a DRAM staging
    buffer, and gathered back into the [120, 2*256] kval tile with mixed-radix
    strided DMAs (DRAM-side access patterns are unconstrained), one DMA per
    (stag-tile, slab) covering both fields, pipelined against later chunks.
  - kval is masked with the host-computed window indicator and contracted
    against integT on the PE, then pushed through sigmoid.
"""

import numpy as np
from contextlib import ExitStack

import concourse.bacc as bacc
import concourse.bass as bass
import concourse.tile as tile
from concourse import mybir
from concourse.bass_utils import run_bass_kernel_spmd

F32 = mybir.dt.float32
F16 = mybir.dt.float16
AF = mybir.ActivationFunctionType
OP = mybir.AluOpType

B, H, N, F, KH = 64, 256, 2048, 2, 20
K = 400
NCORES = 8
NLOC = N // NCORES          # 256 grid points per core
W = 120                     # k-window rows per core (3 slabs x 40)
S = 3                       # packed k-slabs
JW = 40                     # j (within-slab row) count; slab s row = 40s+j
NCOLS = JW * NLOC           # 10240 pair columns = 20 chunks exactly
NCHUNK = 20
FILT = 0.15

# f16 constant blob column offsets
C_W1P, C_W2P, C_W3P = 0, 120, 243
C_FFW1A, C_FFW1B, C_FFW2 = 275, 395, 515
C_FFW3A, C_FFW3B = 755, 875
C_WT0, C_WT1 = 995, 1059
C16 = 1123

LAST_RESULTS = None          # BassKernelResults of the most recent run


def _build_nc():
    nc = bacc.Bacc("TRN2", name="decoder")

    # ---- IO ----
    d_rhs = nc.dram_tensor("rhs", [6, NCOLS], F16, kind="ExternalInput")
    d_b32 = nc.dram_tensor("b32", [8, 136], F32, kind="ExternalInput")
    d_msk = nc.dram_tensor("msk", [W, F * NLOC], F16, kind="ExternalInput")
    d_c16 = nc.dram_tensor("c16", [128, C16], F16, kind="ExternalInput")
    d_out = nc.dram_tensor("out", [B, NLOC, F], F32, kind="ExternalOutput")

    with tile.TileContext(nc) as tc, ExitStack() as ctx:
        consts = ctx.enter_context(tc.tile_pool(name="consts", bufs=1))
        persist = ctx.enter_context(tc.tile_pool(name="persist", bufs=1))
        big = ctx.enter_context(tc.tile_pool(name="big", bufs=1))
        work = ctx.enter_context(tc.tile_pool(name="work", bufs=4))
        kvpool = ctx.enter_context(tc.tile_pool(name="kv", bufs=4))
        dramp = ctx.enter_context(tc.tile_pool(name="dramp", bufs=2, space="DRAM"))
        psum = ctx.enter_context(tc.tile_pool(name="psum", bufs=1, space="PSUM"))

        # preload the activation table (Tanh set also carries Relu/Sigmoid/
        # Identity) so no mid-kernel ~1.3us ACT_TABLE_LOAD fires
        actdum = consts.tile([1, 2], F32, tag="actdum")
        nc.vector.memset(actdum[0:1, 0:1], 0.0)
        nc.scalar.activation(actdum[0:1, 1:2], actdum[0:1, 0:1], AF.Tanh)

        # ---- load inputs (scalar/gpsimd queues spray packets across many
        # DMA engines; sync's static queue only has two, so it gets the
        # small/late transfers).  rhs holds only the 6 meaningful rows and is
        # double-loaded into both L1 row strips; rows 6:32 are never read.
        rhs1 = big.tile([38, NCOLS], F16, tag="rhs1")
        c16 = consts.tile([128, C16], F16, tag="c16")
        rsplit = [0, 2 * 512, 7 * 512, 13 * 512, NCOLS]
        nc.scalar.dma_start(out=rhs1[0:6, 0:rsplit[1]], in_=d_rhs[:, 0:rsplit[1]])
        nc.gpsimd.dma_start(out=rhs1[32:38, 0:rsplit[1]], in_=d_rhs[:, 0:rsplit[1]])
        nc.scalar.dma_start(out=c16[:], in_=d_c16[:, :])
        for p in range(1, 4):
            nc.gpsimd.dma_start(out=rhs1[0:6, rsplit[p]:rsplit[p + 1]],
                                in_=d_rhs[:, rsplit[p]:rsplit[p + 1]])
            nc.scalar.dma_start(out=rhs1[32:38, rsplit[p]:rsplit[p + 1]],
                                in_=d_rhs[:, rsplit[p]:rsplit[p + 1]])
        msk = consts.tile([W, F * NLOC], F16, tag="msk")

        # biases arrive transposed [8, 128] f32 (8 fat DMA packets instead of
        # 128 tiny ones) and get partition-transposed on the PE via the
        # appended 8x8 identity
        b32r = consts.tile([8, 136], F32, tag="b32r")
        nc.gpsimd.dma_start(out=b32r[:], in_=d_b32[:, :])
        psb32 = psum.tile([128, 8], F32, tag="ps3", bufs=2, name="psb32")
        nc.tensor.matmul(psb32[:], b32r[:, 0:128], b32r[:, 128:136],
                         start=True, stop=True)
        c32 = consts.tile([128, 8], F32, tag="c32")
        nc.vector.tensor_copy(c32[:], psb32[:])

        ffb3c = c32[:, 2:3]
        ffb1c = c32[0:120, 3:4]
        ffb2c = c32[0:120, 4:5]
        ffb2d = c32[0:120, 5:6]
        b1p = c32[0:120, 6:7]
        b2p = c32[0:123, 7:8]

        w2p = c16[0:120, C_W2P:C_W2P + 123]
        w3p = c16[0:123, C_W3P:C_W3P + 32]
        ffw1a = c16[0:128, C_FFW1A:C_FFW1A + 120]
        ffw1b = c16[0:128, C_FFW1B:C_FFW1B + 120]
        ffw2 = c16[0:120, C_FFW2:C_FFW2 + 240]
        ffw3a = c16[0:120, C_FFW3A:C_FFW3A + W]
        ffw3b = c16[0:120, C_FFW3B:C_FFW3B + W]
        wt0 = c16[0:128, C_WT0:C_WT0 + 64]
        wt1 = c16[0:128, C_WT1:C_WT1 + 64]

        # ---- FF MLP pieces (emitted interleaved with pair-MLP chunks) ----
        h1ff = work.tile([120, B], F16, tag="h1ff")
        h2ff = work.tile([120, 2 * B], F16, tag="h2ff")
        integT = persist.tile([128, B], F16, tag="integT")

        def ff_l1():
            ps = psum.tile([120, B], F32, tag="ps2", bufs=2, name="ffp")
            nc.tensor.matmul(ps[:], ffw1a, wt0, start=True, stop=False)
            nc.tensor.matmul(ps[:], ffw1b, wt1, start=False, stop=True)
            nc.scalar.activation(h1ff[:], ps[:], AF.Tanh, bias=ffb1c)
            return ps

        def ff_l2():
            ps = psum.tile([120, 2 * B], F32, tag="ps2", bufs=2, name="ffp")
            nc.tensor.matmul(ps[:, 0:B], ffw2[:, 0:120], h1ff[:],
                             start=True, stop=True)
            nc.tensor.matmul(ps[:, B:2 * B], ffw2[:, 120:240], h1ff[:],
                             start=True, stop=True)
            nc.scalar.activation(h2ff[:, 0:B], ps[:, 0:B], AF.Tanh, bias=ffb2c)
            nc.scalar.activation(h2ff[:, B:2 * B], ps[:, B:2 * B], AF.Tanh,
                                 bias=ffb2d)
            return ps

        def ff_l3():
            ps = psum.tile([120, B], F32, tag="ps2", bufs=2, name="ffp")
            nc.tensor.matmul(ps[:], ffw3a, h2ff[:, 0:B], start=True, stop=False)
            nc.tensor.matmul(ps[:], ffw3b, h2ff[:, B:2 * B], start=False, stop=True)
            nc.scalar.activation(integT[0:120, :], ps[:], AF.Identity,
                                 bias=ffb3c[0:120, :])
            return ps

        ff_stages = [ff_l1, ff_l2, ff_l3]

        # ---- pipelined 3-layer pair MLP over 40 256-column units ----
        # 256-col granularity keeps the dependency chain (L1 -> relu -> L2 ->
        # relu -> L3) shorter than 4 units of PE work, so the PE queue never
        # stalls and the tensor engine ramps to its full clock.  Relu work is
        # split across ScalarE/VectorE with one op per engine per step.
        # Layer-3 outputs stack 8 units deep in a PSUM tile via tile_position,
        # are copied once per tile to SBUF (split halves on both engines),
        # bounced into DRAM staging, and gathered per-tile into kval with
        # mixed-radix strided DMAs while later units still run.
        stag = dramp.tile([5, 128, 512], F16, tag="stag")
        kval = persist.tile([W, F * NLOC], F16, tag="kval")
        NU = 2 * NCHUNK
        ps1s, ps2s, ps3s = {}, {}, {}
        g_engines = (nc.sync, nc.sync, nc.sync, nc.gpsimd, nc.gpsimd, nc.gpsimd)

        ps1t, ps2t = {}, {}

        def emit_l1(u):
            if u % 8 == 0:
                ps3s[u // 8] = psum.tile([128, 512], F32, tag="ps3",
                                         bufs=2, name="ps3")
            csl = slice(u * 256, (u + 1) * 256)
            # PSUM banks are 2KB: allocate one [*, 512] bank per unit pair
            # and hand out 256-col halves
            if u % 2 == 0:
                ps1t[u // 2] = psum.tile([120, 512], F32, tag="ps1", bufs=4, name="ps1t")
            ps1 = ps1t[u // 2][:, 256 * (u % 2):256 * (u % 2) + 256]
            r = 32 * ((u // 2) % 2)   # row-strip: L1's K is tiny, so odd/even
            nc.tensor.matmul(ps1, c16[r:r + 6, C_W1P:C_W1P + 120],
                             rhs1[r:r + 6, csl],
                             start=True, stop=True, tile_position=(r, 0))
            ps1s[u] = ps1

        def emit_l2(u):
            ps1 = ps1s.pop(u)
            h1 = work.tile([120, 256], F16, tag="h1")
            if u % 2 == 0:
                nc.scalar.activation(h1[:], ps1, AF.Relu, bias=b1p)
            else:
                nc.vector.tensor_scalar(h1[:], ps1, b1p, 0.0, OP.add, OP.max)
            if u % 2 == 0:
                ps2t[u // 2] = psum.tile([123, 512], F32, tag="ps2", bufs=2, name="ps2t")
            ps2 = ps2t[u // 2][:, 256 * (u % 2):256 * (u % 2) + 256]
            nc.tensor.matmul(ps2, w2p, h1[:], start=True, stop=True)
            ps2s[u] = ps2

        def emit_relu2(u):
            ps2 = ps2s.pop(u)
            h2 = work.tile([123, 256], F16, tag="h2")
            if u % 2 == 0:
                nc.vector.tensor_scalar(h2[:], ps2, b2p, 0.0, OP.add, OP.max)
            else:
                nc.scalar.activation(h2[:], ps2, AF.Relu, bias=b2p)
            ps2s[u] = h2

        def emit_l3(u):
            h2 = ps2s.pop(u)
            ch, hf = divmod(u, 2)
            t, q = divmod(ch, 4)
            nc.tensor.matmul(ps3s[t][32 * q:32 * q + 32, 256 * hf:256 * hf + 256],
                             w3p, h2[:], start=True, stop=True,
                             tile_position=(0, 32 * q))
            if u % 8 == 7:
                kvp = kvpool.tile([128, 512], F16, tag="kvp")
                ps3 = ps3s.pop(t)
                nc.scalar.copy(kvp[0:64, :], ps3[0:64, :])
                nc.vector.tensor_copy(kvp[64:128, :], ps3[64:128, :])
                nc.gpsimd.dma_start(out=stag[t, :, :], in_=kvp[:])
                emit_gather(8 * t, 8 * t + 8)

        def emit_gather(j0, j1):
            # col c = j*256+n -> chunk j//2 at strip 32*((j//2)%4), pos
            # (j%2)*256+n; stag is chunk-linear (chunk ch at 16384*ch), so a
            # run of tiles gathers with one 3-dim AP per (slab, field)
            st = stag[:]
            for sb in range(S):
                for f in range(F):
                    src = bass.AP(
                        tensor=st.tensor,
                        offset=st.offset + 16384 * (j0 // 2) + 512 * (2 * sb + f),
                        ap=[[16384, (j1 - j0) // 2], [256, 2], [1, NLOC]])
                    g_engines[(2 * sb + f) % len(g_engines)].dma_start(
                        out=kval[JW * sb + j0:JW * sb + j1,
                                 f * NLOC:(f + 1) * NLOC],
                        in_=src)

        nc.gpsimd.dma_start(out=msk[:], in_=d_msk[:, :])

        # unit pipeline: the PE runs L1(u), L2(u-2), L3(u-4) each step while
        # the relus of the units in between complete on ScalarE/VectorE; FF
        # stages slot into the PE queue at wide spacings
        for step in range(NU + 4):
            if step < NU:
                emit_l1(step)
            if step in (2, 12, 22):
                ff_stages[(step - 2) // 10]()
            if 2 <= step and step - 2 < NU:
                emit_l2(step - 2)
            if 3 <= step and step - 3 < NU:
                emit_relu2(step - 3)
            if 4 <= step and step - 4 < NU:
                emit_l3(step - 4)

        # ---- mask, contract against integT, sigmoid, store ----
        outsb = persist.tile([B, NLOC, F], F32, tag="outsb")
        nc.vector.tensor_tensor(kval[:], kval[:], msk[:], OP.mult)
        for f in range(F):
            fsl = slice(f * NLOC, (f + 1) * NLOC)
            psF = psum.tile([B, NLOC], F32, tag="ps2", bufs=2)
            nc.tensor.matmul(psF[:], integT[0:W, :], kval[:, fsl],
                             start=True, stop=True)
            nc.scalar.activation(outsb[:, :, f], psF[:], AF.Sigmoid)
        nc.sync.dma_start(out=d_out[:, :, :], in_=outsb[:])

    nc.finalize()
    return nc


_NC_CACHE = None


def _get_nc():
    global _NC_CACHE
    if _NC_CACHE is None:
        _NC_CACHE = _build_nc()
    return _NC_CACHE


def _pack_shared(w):
    """Host-side packing of the grid-independent constants."""
    f32, f16 = np.float32, np.float16
    k_w1, k_b1 = w["k_w1"].astype(f32), w["k_b1"].astype(f32)
    k_w2, k_b2 = w["k_w2"].astype(f32), w["k_b2"].astype(f32)
    k_w3, k_b3 = w["k_w3"].astype(f32), w["k_b3"].astype(f32)
    w1p = np.zeros((38, 120), f32)
    b1p = np.zeros((120,), f32)
    w2p = np.zeros((120, 123), f32)
    b2p = np.zeros((123,), f32)
    w3p = np.zeros((123, 32), f32)
    for s in range(S):
        for f in range(F):
            o = s * 40 + f * 20
            for d in range(2):
                w1p[2 * s + d, o:o + 20] = k_w1[f, d]
                w1p[32 + 2 * s + d, o:o + 20] = k_w1[f, d]
            b1p[o:o + 20] = k_b1[f]
            w2p[o:o + 20, s * 41 + f * 20:s * 41 + f * 20 + 20] = k_w2[f]
            b2p[s * 41 + f * 20:s * 41 + f * 20 + 20] = k_b2[f]
            w3p[s * 41 + f * 20:s * 41 + f * 20 + 20, s * 2 + f] = k_w3[f, :, 0]
            w3p[s * 41 + 40, s * 2 + f] = k_b3[f, 0]
        b2p[s * 41 + 40] = 1.0

    c16 = np.zeros((128, C16), f16)
    c16[0:38, C_W1P:C_W1P + 120] = w1p.astype(f16)
    c16[0:120, C_W2P:C_W2P + 123] = w2p.astype(f16)
    c16[0:123, C_W3P:C_W3P + 32] = w3p.astype(f16)
    ffw1 = w["ff_w1"].astype(f16)
    c16[0:128, C_FFW1A:C_FFW1A + 120] = ffw1[0:128]
    c16[0:128, C_FFW1B:C_FFW1B + 120] = ffw1[128:256]
    c16[0:120, C_FFW2:C_FFW2 + 240] = w["ff_w2"].astype(f16)
    wT = np.ascontiguousarray(w["weights"].astype(f32).T).astype(f16)
    c16[0:128, C_WT0:C_WT0 + 64] = wT[0:128]
    c16[0:128, C_WT1:C_WT1 + 64] = wT[128:256]

    b32 = np.zeros((8, 136), f32)
    b32[3, 0:120] = w["ff_b1"].astype(f32)
    b32[4, 0:120] = w["ff_b2"].astype(f32)[0:120]
    b32[5, 0:120] = w["ff_b2"].astype(f32)[120:240]
    b32[6, 0:120] = b1p
    b32[7, 0:123] = b2p
    b32[:, 128:136] = np.eye(8, dtype=f32)
    return c16, b32


def kernel(**inputs):
    global LAST_RESULTS
    f32, f16 = np.float32, np.float16
    nc = _get_nc()
    c16s, b32s = _pack_shared(inputs)
    grid = inputs["grid"].astype(f32)

    # exact fp32 active-window bases per point (replicates the reference's
    # fp32 center table and window comparisons bit-for-bit)
    g = (np.arange(20, dtype=f32) * f32(0.05)).astype(f32)
    lx = grid[:, 0:1] - g[None, :]   # [N, 20] exact fp32
    ly = grid[:, 1:2] - g[None, :]
    ax = (lx >= 0) & (lx <= f32(FILT))
    ay = (ly >= 0) & (ly <= f32(FILT))
    bx = np.minimum(ax.argmax(1), 16)
    by = np.minimum(ay.argmax(1), 16)
    ii = np.arange(20)[None, :]
    assert np.all(~ax | ((ii >= bx[:, None]) & (ii <= bx[:, None] + 3)))
    assert np.all(~ay | ((ii >= by[:, None]) & (ii <= by[:, None] + 3)))
    k0 = 20 * bx + by
    perm = np.argsort(k0, kind="stable")
    k0s = k0[perm]

    # padded ffw3/ffb3 for per-core window slicing
    ffw3p = np.zeros((240, 512), f32)
    ffw3p[:, :K] = inputs["ff_w3"].astype(f32)
    ffb3p = np.zeros((512,), f32)
    ffb3p[:K] = inputs["ff_b3"].astype(f32)

    in_maps = []
    for c in range(NCORES):
        sl = perm[c * NLOC:(c + 1) * NLOC]
        c0 = int(k0s[c * NLOC])
        assert int(k0s[(c + 1) * NLOC - 1]) - c0 <= W - 64, "window overflow"
        kk = c0 + np.arange(W)
        kix = np.minimum(kk // 20, 19)
        kiy = kk % 20
        # window-local fp32 coords [W, NLOC] (exact: same subtract as ref)
        wlx = grid[sl, 0][None, :] - g[kix][:, None]
        wly = grid[sl, 1][None, :] - g[kiy][:, None]
        inside = ((wlx >= 0) & (wlx <= f32(FILT)) &
                  (wly >= 0) & (wly <= f32(FILT)) &
                  (kk < K)[:, None])
        # pair-MLP rhs [38, 40*256] fp16: col (j, n), slab s rows 2s:2s+2,
        # replicated at 32+2s for the odd-chunk row strip
        rhs = np.zeros((6, NCOLS), f16)
        for s in range(S):
            rhs[2 * s] = wlx[JW * s:JW * s + JW].astype(f16).reshape(-1)
            rhs[2 * s + 1] = wly[JW * s:JW * s + JW].astype(f16).reshape(-1)
        c16 = c16s.copy()
        c16[0:120, C_FFW3A:C_FFW3A + W] = ffw3p[0:120, c0:c0 + W].astype(f16)
        c16[0:120, C_FFW3B:C_FFW3B + W] = ffw3p[120:240, c0:c0 + W].astype(f16)
        b32 = b32s.copy()
        b32[2, 0:W] = ffb3p[c0:c0 + W]
        in_maps.append(dict(
            c16=c16, b32=b32, rhs=rhs,
            msk=np.concatenate([inside.astype(f16)] * F, axis=1),
        ))
    res = run_bass_kernel_spmd(nc, in_maps, core_ids=list(range(NCORES)))
    LAST_RESULTS = res
    out_sorted = np.concatenate([r["out"] for r in res.results], axis=1)
    out = np.empty_like(out_sorted)
    out[:, perm, :] = out_sorted
    return out


# revision 22
# speedup vs baseline: 1.2089x; 1.0647x over previous
"""Trainium2 Bass kernel for nn_Decoder_1692217114985 (continuous transpose-conv decoder).

Math (see the reference):
  integ = FF(weights)                         # [B=64, K=400] per-stride integrals
  kval[f,n,k] = MLP_f(grid[n] - center[k])    # masked to the 0.15-window
  out = sigmoid(einsum('fnk,bk->bnf', kval, integ))

Key optimization over the dense baseline: the 0.15 filter window only spans
3-4 center strides per axis, so each grid point's active centers live in a
64-wide contiguous band of k = 20*ix+iy indices starting at k0 = 20*bx+by.
The host sorts points by k0 and gives each core 256 points whose bands all
fit in ONE 120-row k-window [c0, c0+120).  Each core then runs the dense
pair-MLP against only its 120 window rows (30k pairs) instead of all 400
centers (102k pairs), with per-core-sliced ffw3/ffb3 and host-precomputed
MLP inputs making the program pure SPMD.  Outputs are un-permuted on the
host.

Per-core layout:
  - The pair-MLP rhs (window-local coords, fp16) and the exact fp32 window
    mask are pure functions of `grid`, so the host precomputes both; the
    device spends no time on coordinate broadcasts or mask compares.
  - All matmul datapaths run in float16; PSUM accumulation is fp32.
  - FF MLP computed transposed (features on partitions, batch on free dim)
    producing integT [120(k-window), 64] directly from the sliced ffw3, and
    is interleaved with the early pair-MLP chunks on the PE queue.
  - The pair MLP (2->20->20->1, x2 fields) is evaluated over the window with
    a 3-way block-diagonal packing: 3 k-slabs of 40 rows stacked on the
    contraction dim, so each PE column evaluates 3 (point, center) pairs.
    Columns are (j, n) with j in [0,40), n in [0,256): slab s row = 40s+j.
  - Layer-2 weights carry an extra constant-1 unit per slice so the layer-3
    bias rides through the matmul; layer-1/2 biases enter via the per-partition
    bias ports (relu work alternates ScalarE/VectorE).
  - Layer-3 outputs are stacked 4 chunks per PSUM tile via tile_position=
    (0,32q), copied once per tile to SBUF, bounced through a DRAM staging
    buffer, and gathered back into the [120, 2*256] kval tile with mixed-radix
    strided DMAs (DRAM-side access patterns are unconstrained), one DMA per
    (stag-tile, slab) covering both fields, pipelined against later chunks.
  - kval is masked with the host-computed window indicator and contracted
    against integT on the PE, then pushed through sigmoid.
"""

import numpy as np
from contextlib import ExitStack

import concourse.bacc as bacc
import concourse.bass as bass
import concourse.tile as tile
from concourse import mybir
from concourse.bass_utils import run_bass_kernel_spmd

F32 = mybir.dt.float32
F16 = mybir.dt.float16
AF = mybir.ActivationFunctionType
OP = mybir.AluOpType

B, H, N, F, KH = 64, 256, 2048, 2, 20
K = 400
NCORES = 8
NLOC = N // NCORES          # 256 grid points per core
W = 120                     # k-window rows per core (3 slabs x 40)
S = 3                       # packed k-slabs
JW = 40                     # j (within-slab row) count; slab s row = 40s+j
NCOLS = JW * NLOC           # 10240 pair columns = 20 chunks exactly
NCHUNK = 20
FILT = 0.15

# f16 constant blob column offsets
C_W1P, C_W2P, C_W3P = 0, 120, 243
C_FFW1A, C_FFW1B, C_FFW2 = 275, 395, 515
C_FFW3A, C_FFW3B = 755, 875
C_WT0, C_WT1 = 995, 1059
C16 = 1123

LAST_RESULTS = None          # BassKernelResults of the most recent run


def _build_nc():
    nc = bacc.Bacc("TRN2", name="decoder")

    # ---- IO ----
    d_rhs = nc.dram_tensor("rhs", [6, NCOLS], F16, kind="ExternalInput")
    d_b32 = nc.dram_tensor("b32", [8, 136], F32, kind="ExternalInput")
    d_msk = nc.dram_tensor("msk", [W, F * NLOC], F16, kind="ExternalInput")
    d_c16 = nc.dram_tensor("c16", [128, C16], F16, kind="ExternalInput")
    d_out = nc.dram_tensor("out", [B, NLOC, F], F32, kind="ExternalOutput")

    with tile.TileContext(nc) as tc, ExitStack() as ctx:
        consts = ctx.enter_context(tc.tile_pool(name="consts", bufs=1))
        persist = ctx.enter_context(tc.tile_pool(name="persist", bufs=1))
        big = ctx.enter_context(tc.tile_pool(name="big", bufs=1))
        work = ctx.enter_context(tc.tile_pool(name="work", bufs=4))
        kvpool = ctx.enter_context(tc.tile_pool(name="kv", bufs=4))
        dramp = ctx.enter_context(tc.tile_pool(name="dramp", bufs=2, space="DRAM"))
        psum = ctx.enter_context(tc.tile_pool(name="psum", bufs=1, space="PSUM"))

        # preload the activation table (Tanh set also carries Relu/Sigmoid/
        # Identity) so no mid-kernel ~1.3us ACT_TABLE_LOAD fires
        actdum = consts.tile([1, 2], F32, tag="actdum")
        nc.vector.memset(actdum[0:1, 0:1], 0.0)
        nc.scalar.activation(actdum[0:1, 1:2], actdum[0:1, 0:1], AF.Sigmoid)

        # ---- load inputs (scalar/gpsimd queues spray packets across many
        # DMA engines; sync's static queue only has two, so it gets the
        # small/late transfers).  rhs holds only the 6 meaningful rows and is
        # double-loaded into both L1 row strips; rows 6:32 are never read.
        rhs1 = big.tile([38, NCOLS], F16, tag="rhs1")
        c16 = consts.tile([128, C16], F16, tag="c16")
        rsplit = [0, 2 * 512, 7 * 512, 13 * 512, NCOLS]
        nc.scalar.dma_start(out=c16[:], in_=d_c16[:, :])
        nc.gpsimd.dma_start(out=rhs1[0:6, 0:rsplit[1]], in_=d_rhs[:, 0:rsplit[1]])
        nc.scalar.dma_start(out=rhs1[32:38, 0:rsplit[1]], in_=d_rhs[:, 0:rsplit[1]])
        for p in range(1, 4):
            nc.gpsimd.dma_start(out=rhs1[0:6, rsplit[p]:rsplit[p + 1]],
                                in_=d_rhs[:, rsplit[p]:rsplit[p + 1]])
            nc.scalar.dma_start(out=rhs1[32:38, rsplit[p]:rsplit[p + 1]],
                                in_=d_rhs[:, rsplit[p]:rsplit[p + 1]])
        msk = consts.tile([W, F * NLOC], F16, tag="msk")

        # biases arrive transposed [8, 128] f32 (8 fat DMA packets instead of
        # 128 tiny ones) and get partition-transposed on the PE via the
        # appended 8x8 identity
        b32r = consts.tile([8, 136], F32, tag="b32r")
        nc.gpsimd.dma_start(out=b32r[:], in_=d_b32[:, :])
        psb32 = psum.tile([128, 8], F32, tag="ps3", bufs=2, name="psb32")
        nc.tensor.matmul(psb32[:], b32r[:, 0:128], b32r[:, 128:136],
                         start=True, stop=True)
        c32 = consts.tile([128, 8], F32, tag="c32")
        nc.vector.tensor_copy(c32[:], psb32[:])

        ffb3c = c32[:, 2:3]
        ffb1c = c32[0:120, 3:4]
        ffb2c = c32[0:120, 4:5]
        ffb2d = c32[0:120, 5:6]
        b1p = c32[0:120, 6:7]
        b2p = c32[0:123, 7:8]

        w2p = c16[0:120, C_W2P:C_W2P + 123]
        w3p = c16[0:123, C_W3P:C_W3P + 32]
        ffw1a = c16[0:128, C_FFW1A:C_FFW1A + 120]
        ffw1b = c16[0:128, C_FFW1B:C_FFW1B + 120]
        ffw2 = c16[0:120, C_FFW2:C_FFW2 + 240]
        ffw3a = c16[0:120, C_FFW3A:C_FFW3A + W]
        ffw3b = c16[0:120, C_FFW3B:C_FFW3B + W]
        wt0 = c16[0:128, C_WT0:C_WT0 + 64]
        wt1 = c16[0:128, C_WT1:C_WT1 + 64]

        # ---- FF MLP pieces (emitted interleaved with pair-MLP chunks) ----
        h1ff = work.tile([120, B], F16, tag="h1ff")
        h2ff = work.tile([120, 2 * B], F16, tag="h2ff")
        integT = persist.tile([128, B], F16, tag="integT")

        def ff_l1():
            ps = psum.tile([120, B], F32, tag="ps2", bufs=2, name="ffp")
            nc.tensor.matmul(ps[:], ffw1a, wt0, start=True, stop=False)
            nc.tensor.matmul(ps[:], ffw1b, wt1, start=False, stop=True)
            nc.scalar.activation(h1ff[:], ps[:], AF.Tanh, bias=ffb1c)
            return ps

        def ff_l2():
            ps = psum.tile([120, 2 * B], F32, tag="ps2", bufs=2, name="ffp")
            nc.tensor.matmul(ps[:, 0:B], ffw2[:, 0:120], h1ff[:],
                             start=True, stop=True)
            nc.tensor.matmul(ps[:, B:2 * B], ffw2[:, 120:240], h1ff[:],
                             start=True, stop=True)
            nc.scalar.activation(h2ff[:, 0:B], ps[:, 0:B], AF.Tanh, bias=ffb2c)
            nc.scalar.activation(h2ff[:, B:2 * B], ps[:, B:2 * B], AF.Tanh,
                                 bias=ffb2d)
            return ps

        def ff_l3():
            ps = psum.tile([120, B], F32, tag="ps2", bufs=2, name="ffp")
            nc.tensor.matmul(ps[:], ffw3a, h2ff[:, 0:B], start=True, stop=False)
            nc.tensor.matmul(ps[:], ffw3b, h2ff[:, B:2 * B], start=False, stop=True)
            nc.scalar.activation(integT[0:120, :], ps[:], AF.Identity,
                                 bias=ffb3c[0:120, :])
            return ps

        ff_stages = [ff_l1, ff_l2, ff_l3]

        # ---- pipelined 3-layer pair MLP over 40 256-column units ----
        # 256-col granularity keeps the dependency chain (L1 -> relu -> L2 ->
        # relu -> L3) shorter than 4 units of PE work, so the PE queue never
        # stalls and the tensor engine ramps to its full clock.  Relu work is
        # split across ScalarE/VectorE with one op per engine per step.
        # Layer-3 outputs stack 8 units deep in a PSUM tile via tile_position,
        # are copied once per tile to SBUF (split halves on both engines),
        # bounced into DRAM staging, and gathered per-tile into kval with
        # mixed-radix strided DMAs while later units still run.
        stag = dramp.tile([5, 128, 512], F16, tag="stag")
        kval = persist.tile([W, F * NLOC], F16, tag="kval")
        NU = 2 * NCHUNK
        ps1s, ps2s, ps3s = {}, {}, {}
        g_engines = (nc.sync, nc.sync, nc.gpsimd, nc.sync, nc.sync, nc.gpsimd)

        ps1t, ps2t = {}, {}

        def emit_l1(u):
            if u % 8 == 0:
                ps3s[u // 8] = psum.tile([128, 512], F32, tag="ps3",
                                         bufs=2, name="ps3")
            csl = slice(u * 256, (u + 1) * 256)
            # PSUM banks are 2KB: allocate one [*, 512] bank per unit pair
            # and hand out 256-col halves
            if u % 2 == 0:
                ps1t[u // 2] = psum.tile([120, 512], F32, tag="ps1", bufs=4, name="ps1t")
            ps1 = ps1t[u // 2][:, 256 * (u % 2):256 * (u % 2) + 256]
            r = 32 * ((u // 2) % 2)   # row-strip: L1's K is tiny, so odd/even
            nc.tensor.matmul(ps1, c16[r:r + 6, C_W1P:C_W1P + 120],
                             rhs1[r:r + 6, csl],
                             start=True, stop=True, tile_position=(r, 0))
            ps1s[u] = ps1

        def emit_relu1(u):
            ps1 = ps1s.pop(u)
            h1 = work.tile([120, 256], F16, tag="h1")
            if u % 2 == 0:
                nc.scalar.activation(h1[:], ps1, AF.Relu, bias=b1p)
            else:
                nc.vector.tensor_scalar(h1[:], ps1, b1p, 0.0, OP.add, OP.max)
            ps1s[u] = h1

        def emit_l2(u):
            h1 = ps1s.pop(u)
            if u % 2 == 0:
                ps2t[u // 2] = psum.tile([123, 512], F32, tag="ps2", bufs=2,
                                         name="ps2t")
            ps2 = ps2t[u // 2][:, 256 * (u % 2):256 * (u % 2) + 256]
            nc.tensor.matmul(ps2, w2p, h1[:], start=True, stop=True)
            ps2s[u] = ps2

        def emit_relu2(u):
            ps2 = ps2s.pop(u)
            h2 = work.tile([123, 256], F16, tag="h2")
            if u % 2 == 0:
                nc.vector.tensor_scalar(h2[:], ps2, b2p, 0.0, OP.add, OP.max)
            else:
                nc.scalar.activation(h2[:], ps2, AF.Relu, bias=b2p)
            ps2s[u] = h2

        def emit_l3(u):
            h2 = ps2s.pop(u)
            ch, hf = divmod(u, 2)
            t, q = divmod(ch, 4)
            nc.tensor.matmul(ps3s[t][32 * q:32 * q + 32, 256 * hf:256 * hf + 256],
                             w3p, h2[:], start=True, stop=True,
                             tile_position=(0, 32 * q))
            if u % 8 == 7:
                kvp = kvpool.tile([128, 512], F16, tag="kvp")
                ps3 = ps3s.pop(t)
                nc.scalar.copy(kvp[0:64, :], ps3[0:64, :])
                nc.vector.tensor_copy(kvp[64:128, :], ps3[64:128, :])
                nc.gpsimd.dma_start(out=stag[t, :, :], in_=kvp[:])
                emit_gather(t)
                if t == 2:
                    # tiles 0-2 cover krel [0, 72): mask the first 64 rows now
                    # so the contraction's first block is ready right after
                    # the loop (matmul base partitions must be 0/32/64)
                    nc.vector.tensor_tensor(kval[0:64, :], kval[0:64, :],
                                            msk[0:64, :], OP.mult)

        def emit_gather(t):
            # col c = j*256+n -> chunk j//2 at strip 32*((j//2)%4), pos
            # (j%2)*256+n; stag is chunk-linear (chunk ch at 16384*ch).  With
            # the stride-3 slab interleave, slab s row j holds krel = 3j+s, so
            # tile t lands at kval rows [24t, 24t+24) with a stride-3
            # partition pattern per slab.
            st = stag[:]
            kv = kval[:]
            for sb in range(S):
                for f in range(F):
                    src = bass.AP(
                        tensor=st.tensor,
                        offset=st.offset + 16384 * 4 * t + 512 * (2 * sb + f),
                        ap=[[16384, 4], [256, 2], [1, NLOC]])
                    dst = bass.AP(
                        tensor=kv.tensor,
                        offset=kv.offset + (24 * t + sb) * (F * NLOC) + f * NLOC,
                        ap=[[3 * F * NLOC, 8], [1, NLOC]])
                    g_engines[(2 * sb + f) % len(g_engines)].dma_start(
                        out=dst, in_=src)

        nc.gpsimd.dma_start(out=msk[:], in_=d_msk[:, :])

        # unit pipeline, depth 8: the PE runs L1(u), L2(u-4), L3(u-8) each
        # step; relus sit 2 steps after their producer and 2 before their
        # consumer, so every PE wait is satisfied on arrival and the tensor
        # engine streams gaplessly into its full-clock p-state
        for step in range(NU + 8):
            if step < NU:
                emit_l1(step)
            if step in (2, 12, 22):
                ff_stages[(step - 2) // 10]()
            if 2 <= step and step - 2 < NU:
                emit_relu1(step - 2)
            if 4 <= step and step - 4 < NU:
                emit_l2(step - 4)
            if 6 <= step and step - 6 < NU:
                emit_relu2(step - 6)
            if 8 <= step and step - 8 < NU:
                emit_l3(step - 8)

        # ---- tail: mask the last 24 rows, finish the split contraction,
        # sigmoid, store ----
        outsb = persist.tile([B, NLOC, F], F32, tag="outsb")
        nc.vector.tensor_tensor(kval[64:120, :], kval[64:120, :],
                                msk[64:120, :], OP.mult)
        psFs = []
        for f in range(F):
            fsl = slice(f * NLOC, (f + 1) * NLOC)
            psF = psum.tile([B, NLOC], F32, tag="ps2", bufs=2, name="psF")
            nc.tensor.matmul(psF[:], integT[0:64, :], kval[0:64, fsl],
                             start=True, stop=False)
            psFs.append((psF, fsl))
        for f, (psF, fsl) in enumerate(psFs):
            nc.tensor.matmul(psF[:], integT[64:W, :], kval[64:W, fsl],
                             start=False, stop=True)
            nc.scalar.activation(outsb[:, :, f], psF[:], AF.Sigmoid)
        nc.sync.dma_start(out=d_out[:, :, :], in_=outsb[:])

    nc.finalize()
    return nc


_NC_CACHE = None


def _get_nc():
    global _NC_CACHE
    if _NC_CACHE is None:
        _NC_CACHE = _build_nc()
    return _NC_CACHE


def _pack_shared(w):
    """Host-side packing of the grid-independent constants."""
    f32, f16 = np.float32, np.float16
    k_w1, k_b1 = w["k_w1"].astype(f32), w["k_b1"].astype(f32)
    k_w2, k_b2 = w["k_w2"].astype(f32), w["k_b2"].astype(f32)
    k_w3, k_b3 = w["k_w3"].astype(f32), w["k_b3"].astype(f32)
    w1p = np.zeros((38, 120), f32)
    b1p = np.zeros((120,), f32)
    w2p = np.zeros((120, 123), f32)
    b2p = np.zeros((123,), f32)
    w3p = np.zeros((123, 32), f32)
    for s in range(S):
        for f in range(F):
            o = s * 40 + f * 20
            for d in range(2):
                w1p[2 * s + d, o:o + 20] = k_w1[f, d]
                w1p[32 + 2 * s + d, o:o + 20] = k_w1[f, d]
            b1p[o:o + 20] = k_b1[f]
            w2p[o:o + 20, s * 41 + f * 20:s * 41 + f * 20 + 20] = k_w2[f]
            b2p[s * 41 + f * 20:s * 41 + f * 20 + 20] = k_b2[f]
            w3p[s * 41 + f * 20:s * 41 + f * 20 + 20, s * 2 + f] = k_w3[f, :, 0]
            w3p[s * 41 + 40, s * 2 + f] = k_b3[f, 0]
        b2p[s * 41 + 40] = 1.0

    c16 = np.zeros((128, C16), f16)
    c16[0:38, C_W1P:C_W1P + 120] = w1p.astype(f16)
    c16[0:120, C_W2P:C_W2P + 123] = w2p.astype(f16)
    c16[0:123, C_W3P:C_W3P + 32] = w3p.astype(f16)
    ffw1 = w["ff_w1"].astype(f16)
    c16[0:128, C_FFW1A:C_FFW1A + 120] = ffw1[0:128]
    c16[0:128, C_FFW1B:C_FFW1B + 120] = ffw1[128:256]
    c16[0:120, C_FFW2:C_FFW2 + 240] = w["ff_w2"].astype(f16)
    wT = np.ascontiguousarray(w["weights"].astype(f32).T).astype(f16)
    c16[0:128, C_WT0:C_WT0 + 64] = wT[0:128]
    c16[0:128, C_WT1:C_WT1 + 64] = wT[128:256]

    b32 = np.zeros((8, 136), f32)
    b32[3, 0:120] = w["ff_b1"].astype(f32)
    b32[4, 0:120] = w["ff_b2"].astype(f32)[0:120]
    b32[5, 0:120] = w["ff_b2"].astype(f32)[120:240]
    b32[6, 0:120] = b1p
    b32[7, 0:123] = b2p
    b32[:, 128:136] = np.eye(8, dtype=f32)
    return c16, b32


def kernel(**inputs):
    global LAST_RESULTS
    f32, f16 = np.float32, np.float16
    nc = _get_nc()
    c16s, b32s = _pack_shared(inputs)
    grid = inputs["grid"].astype(f32)

    # exact fp32 active-window bases per point (replicates the reference's
    # fp32 center table and window comparisons bit-for-bit)
    g = (np.arange(20, dtype=f32) * f32(0.05)).astype(f32)
    lx = grid[:, 0:1] - g[None, :]   # [N, 20] exact fp32
    ly = grid[:, 1:2] - g[None, :]
    ax = (lx >= 0) & (lx <= f32(FILT))
    ay = (ly >= 0) & (ly <= f32(FILT))
    bx = np.minimum(ax.argmax(1), 16)
    by = np.minimum(ay.argmax(1), 16)
    ii = np.arange(20)[None, :]
    assert np.all(~ax | ((ii >= bx[:, None]) & (ii <= bx[:, None] + 3)))
    assert np.all(~ay | ((ii >= by[:, None]) & (ii <= by[:, None] + 3)))
    k0 = 20 * bx + by
    perm = np.argsort(k0, kind="stable")
    k0s = k0[perm]

    # padded ffw3/ffb3 for per-core window slicing
    ffw3p = np.zeros((240, 512), f32)
    ffw3p[:, :K] = inputs["ff_w3"].astype(f32)
    ffb3p = np.zeros((512,), f32)
    ffb3p[:K] = inputs["ff_b3"].astype(f32)

    in_maps = []
    for c in range(NCORES):
        sl = perm[c * NLOC:(c + 1) * NLOC]
        c0 = int(k0s[c * NLOC])
        assert int(k0s[(c + 1) * NLOC - 1]) - c0 <= W - 64, "window overflow"
        kk = c0 + np.arange(W)
        kix = np.minimum(kk // 20, 19)
        kiy = kk % 20
        # window-local fp32 coords [W, NLOC] (exact: same subtract as ref)
        wlx = grid[sl, 0][None, :] - g[kix][:, None]
        wly = grid[sl, 1][None, :] - g[kiy][:, None]
        inside = ((wlx >= 0) & (wlx <= f32(FILT)) &
                  (wly >= 0) & (wly <= f32(FILT)) &
                  (kk < K)[:, None])
        # pair-MLP rhs [38, 40*256] fp16: col (j, n), slab s rows 2s:2s+2,
        # replicated at 32+2s for the odd-chunk row strip
        rhs = np.zeros((6, NCOLS), f16)
        for s in range(S):
            rhs[2 * s] = wlx[s::3].astype(f16).reshape(-1)
            rhs[2 * s + 1] = wly[s::3].astype(f16).reshape(-1)
        c16 = c16s.copy()
        c16[0:120, C_FFW3A:C_FFW3A + W] = ffw3p[0:120, c0:c0 + W].astype(f16)
        c16[0:120, C_FFW3B:C_FFW3B + W] = ffw3p[120:240, c0:c0 + W].astype(f16)
        b32 = b32s.copy()
        b32[2, 0:W] = ffb3p[c0:c0 + W]
        in_maps.append(dict(
            c16=c16, b32=b32, rhs=rhs,
            msk=np.concatenate([inside.astype(f16)] * F, axis=1),
        ))
    res = run_bass_kernel_spmd(nc, in_maps, core_ids=list(range(NCORES)))
    LAST_RESULTS = res
    out_sorted = np.concatenate([r["out"] for r in res.results], axis=1)
    out = np.empty_like(out_sorted)
    out[:, perm, :] = out_sorted
    return out
